# revision 18
# baseline (speedup 1.0000x reference)
"""CIN (xDeepFM) 3-layer kernel for Trainium2, 8-core data parallel. v7.

Math (per layer l, with IN = input viewed [F=64, n] and X = previous
activation [H, n], n = (b, d) flattened):
    pre[o, n] = sum_{h, f} Wl[o, h, f] * X[h, n] * IN[f, n]
    Xnext = relu(pre + bl);  out_l[o, b] = sum_d Xnext[o, (b, d)]

v7 changes vs v2 (190us):
  - Software-pipelined chunk order: L0(c+1) is emitted between L1(c)
    and L2(c).  In v2 the DVE idles at every layer boundary waiting for
    relu (last z8 -> 8 MMs -> ACT relu -> next layer's first z8); with
    the reorder, L0(c+1)'s independent z-builds (x0-based) fill the
    L1(c)->L2(c) bubble, and x1(c+1) is already computed when L2(c)
    ends, removing the L2(c)->L1(c+1) bubble entirely (~2 bubbles x
    ~2us x 4 chunks per pass).
  - Deeper table prefetch (tabbufs 13) and one more z buffer; the dead
    v2 square-path machinery (iden/insq/sel8/cw1/cw2) is removed to pay
    for the SBUF.
  - Layer 0 keeps the symmetric mod-64 diagonal tiling (K 4096 -> 2112,
    17 k-tiles of which one is K=64) with host-packed tables.
"""

import numpy as np
import ml_dtypes

import concourse.bass as bass
import concourse.bacc as bacc
import concourse.tile as tile
import concourse.mybir as mybir
from concourse.bass_utils import run_bass_kernel_spmd

BF16 = ml_dtypes.bfloat16

B, F, D = 512, 64, 32
NCORES = 8
BL = B // NCORES          # 64 batches per core
N = BL * D                # 2048 columns per core
CH = 512                  # chunk width (columns)
NCH = N // CH             # 4 chunks
O = 128                   # out channels per layer
GRP = 8                   # table rows per slot tile
NSL = F // GRP            # slots per layer 1/2 = 8
NT0 = 16                  # full layer-0 k-tiles (plus one K=64 tail)
bf16 = mybir.dt.bfloat16
f32 = mybir.dt.float32

_cache = {}


def _build_program(bench_repeat=None, zbufs=4, xcbufs=5, tabbufs=13):
    from contextlib import ExitStack, nullcontext

    nc = bacc.Bacc("TRN2")
    inp = nc.declare_dram_parameter("inp", [2 * F, N], bf16, isOutput=False)
    w0 = nc.declare_dram_parameter("w0", [128, NT0, 128], bf16, isOutput=False)
    w0h = nc.declare_dram_parameter("w0h", [64, 128], bf16, isOutput=False)
    w1 = nc.declare_dram_parameter("w1", [128, F, 128], bf16, isOutput=False)
    w2 = nc.declare_dram_parameter("w2", [128, F, 128], bf16, isOutput=False)
    b0 = nc.declare_dram_parameter("b0", [128, 1], f32, isOutput=False)
    b1 = nc.declare_dram_parameter("b1", [128, 1], f32, isOutput=False)
    b2 = nc.declare_dram_parameter("b2", [128, 1], f32, isOutput=False)
    # tables: layer-0 sym tiles (16 full in 2 GRP-slots + one 64-row tail),
    # layer-1/2 f-row slots (shared between the two layers)
    tab0 = nc.declare_dram_parameter("tab0", [NCH, 2, 128, GRP, CH], bf16,
                                     isOutput=False)
    tab0h = nc.declare_dram_parameter("tab0h", [NCH, 64, CH], bf16,
                                      isOutput=False)
    tab1 = nc.declare_dram_parameter("tab1", [NCH, NSL, 128, GRP, CH], bf16,
                                     isOutput=False)
    out = nc.declare_dram_parameter("out", [3, 128, BL], f32, isOutput=True)

    with tile.TileContext(nc) as tc, ExitStack() as ctx:
        wpool = ctx.enter_context(tc.tile_pool(name="w", bufs=1))
        xpool = ctx.enter_context(tc.tile_pool(name="x0", bufs=1))
        xc_pool = ctx.enter_context(tc.tile_pool(name="xc", bufs=xcbufs))
        tabs = ctx.enter_context(tc.tile_pool(name="tabs", bufs=tabbufs))
        zpool = ctx.enter_context(tc.tile_pool(name="z", bufs=zbufs))
        opool = ctx.enter_context(tc.tile_pool(name="oacc", bufs=1))
        pspool = ctx.enter_context(tc.tile_pool(name="ps", bufs=3, space="PSUM"))

        # resident weights / constants.  Load order matters for the
        # single-pass latency: x0 + layer-0 weights first (needed by the
        # first compute), the 4MB of w1/w2 after (first needed ~10us in).
        x0_t = xpool.tile([128, N], bf16)
        nc.sync.dma_start(x0_t[:], inp[:])
        w0_t = wpool.tile([128, NT0, 128], bf16)
        nc.sync.dma_start(w0_t[:], w0[:])
        w0h_t = wpool.tile([64, 128], bf16)
        nc.sync.dma_start(w0h_t[:], w0h[:])
        bias_ts = []
        for nm, bd in (("b0", b0), ("b1", b1), ("b2", b2)):
            bt = wpool.tile([128, 1], f32, name=nm)
            nc.sync.dma_start(bt[:], bd[:])
            bias_ts.append(bt)
        # issue via the ACT engine's DGE: separate DMA queue from the
        # (SP-issued) chunk tables, so table DMAs are not queued behind
        # 4MB of weights at single-pass start
        w1_t = wpool.tile([128, F, 128], bf16)
        nc.scalar.dma_start(w1_t[:], w1[:])
        w2_t = wpool.tile([128, F, 128], bf16)
        nc.scalar.dma_start(w2_t[:], w2[:])

        oacc = [opool.tile([128, BL], f32, name=f"oacc{i}", tag=f"oacc{i}")
                for i in range(3)]

        def load_tables(c):
            t0 = []
            for g in range(2):
                s = tabs.tile([128, GRP, CH], bf16, tag="tab", name="s")
                nc.sync.dma_start(s[:], tab0[c, g])
                t0.append(s)
            t0h = tabs.tile([64, CH], bf16, tag="tabh", name="t0h")
            nc.sync.dma_start(t0h[:], tab0h[c])
            t1 = []
            for g in range(NSL):
                s = tabs.tile([128, GRP, CH], bf16, tag="tab", name="s")
                nc.sync.dma_start(s[:], tab1[c, g])
                t1.append(s)
            return t0, t0h, t1

        def emit_l0(c, tbl):
            t0, t0h, _ = tbl
            ns = c * CH
            bsl = c * (CH // D)
            ps0 = pspool.tile([128, CH], f32, tag="ps", name="ps0")
            for g in range(2):
                z8 = zpool.tile([128, GRP, CH], bf16, tag="z", name="z8")
                nc.vector.tensor_mul(
                    z8[:], x0_t[:, ns:ns + CH].unsqueeze(1)
                    .broadcast_to([128, GRP, CH]), t0[g][:])
                for j in range(GRP):
                    m = g * GRP + j
                    nc.tensor.matmul(ps0[:], w0_t[:, m, :], z8[:, j, :],
                                     start=(m == 0), stop=False)
            zh = zpool.tile([64, CH], bf16, tag="zh", name="zh")
            nc.vector.tensor_mul(zh[:], x0_t[0:64, ns:ns + CH], t0h[:])
            nc.tensor.matmul(ps0[:], w0h_t[:], zh[:], start=False, stop=True)

            x1c = xc_pool.tile([128, CH], bf16, tag="xc", name="x1c")
            nc.scalar.activation(x1c[:], ps0[:],
                                 mybir.ActivationFunctionType.Relu,
                                 bias=bias_ts[0], scale=1.0)
            nc.vector.tensor_reduce(
                oacc[0][:, bsl:bsl + CH // D],
                x1c.rearrange("p (g d) -> p g d", d=D),
                axis=mybir.AxisListType.X, op=mybir.AluOpType.add)
            return x1c

        def emit_layer(li, c, xin, tbl):
            t1 = tbl[2]
            bsl = c * (CH // D)
            w_t = w1_t if li == 1 else w2_t
            ps = pspool.tile([128, CH], f32, tag="ps", name="ps")
            for g in range(NSL):
                z8 = zpool.tile([128, GRP, CH], bf16, tag="z", name="z8")
                nc.vector.tensor_mul(
                    z8[:], xin[:].unsqueeze(1)
                    .broadcast_to([128, GRP, CH]), t1[g][:])
                for j in range(GRP):
                    f = g * GRP + j
                    nc.tensor.matmul(ps[:], w_t[:, f, :], z8[:, j, :],
                                     start=(f == 0), stop=(f == F - 1))
            xo = xc_pool.tile([128, CH], bf16, tag="xc", name="xo")
            nc.scalar.activation(xo[:], ps[:],
                                 mybir.ActivationFunctionType.Relu,
                                 bias=bias_ts[li], scale=1.0)
            nc.vector.tensor_reduce(
                oacc[li][:, bsl:bsl + CH // D],
                xo.rearrange("p (g d) -> p g d", d=D),
                axis=mybir.AxisListType.X, op=mybir.AluOpType.add)
            return xo

        loop_cm = tc.For_i(0, bench_repeat, 1) if bench_repeat else nullcontext()
        with loop_cm:
            tbl = load_tables(0)
            x1 = emit_l0(0, tbl)
            for c in range(NCH):
                tbl_next = load_tables(c + 1) if c + 1 < NCH else None
                x2 = emit_layer(1, c, x1, tbl)
                # L0(c+1) between L1(c) and L2(c): its x0-based z-builds
                # keep the DVE busy during L1(c)'s MM/relu tail, and
                # x1(c+1) is ready before L2(c) ends
                x1 = emit_l0(c + 1, tbl_next) if tbl_next else None
                emit_layer(2, c, x2, tbl)
                tbl = tbl_next

            for li in range(3):
                nc.sync.dma_start(out[li], oacc[li][:])

    nc.finalize()
    return nc


def _pack_weights(W0, b0, W1, b1, W2, b2):
    O_, F_ = 128, 64
    W0r = np.asarray(W0, np.float32).reshape(O_, F_, F_)   # [o, h, f]
    SW0 = W0r + W0r.transpose(0, 2, 1)

    # layer 0: tile m (0..15) packs groups t=2m (p<64) and t=2m+1 (p>=64);
    # tail tile = group t=32 at half weight. weight[p, m, o].
    a = np.arange(64)
    w0p = np.empty((128, NT0, O_), np.float32)
    for m in range(NT0):
        for half, t in ((0, 2 * m), (1, 2 * m + 1)):
            f = (a + t) % 64
            wv = SW0[:, a, f]                    # [o, 64]
            if t == 0:
                wv = wv / 2                      # diag counted twice in SW0
            w0p[half * 64:half * 64 + 64, m, :] = wv.T
    fh = (a + 32) % 64
    w0h = (SW0[:, a, fh] / 2).T                  # [64, o]

    def pack_l(W):
        Wr = np.asarray(W, np.float32).reshape(O_, 128, F_)   # [o, h, f]
        return np.ascontiguousarray(Wr.transpose(1, 2, 0)).astype(BF16)

    return {
        "w0": w0p.astype(BF16), "w0h": w0h.astype(BF16),
        "w1": pack_l(W1), "w2": pack_l(W2),
        "b0": np.asarray(b0, np.float32).reshape(128, 1),
        "b1": np.asarray(b1, np.float32).reshape(128, 1),
        "b2": np.asarray(b2, np.float32).reshape(128, 1),
    }


def make_in_maps(input, W0, b0, W1, b1, W2, b2):
    shared = _pack_weights(W0, b0, W1, b1, W2, b2)
    a = np.arange(64)
    in_maps = []
    inp_np = np.asarray(input)
    for core in range(NCORES):
        shard = inp_np[core * BL:(core + 1) * BL]          # [BL, F, D]
        IN = np.ascontiguousarray(
            shard.transpose(1, 0, 2).reshape(F, N)).astype(BF16)
        INs = np.ascontiguousarray(np.concatenate([IN, IN], axis=0))
        INf = IN.reshape(F, NCH, CH)
        # layer-0 sym tables: tab0[c, g, p, j, n] = IN[(p%64 + t)%64, ...],
        # t = 2*(8g+j) + p//64
        t0a = np.empty((NCH, 2, 128, GRP, CH), BF16)
        for g in range(2):
            for j in range(GRP):
                m = g * GRP + j
                t0a[:, g, 0:64, j, :] = np.transpose(
                    INf[(a + 2 * m) % 64], (1, 0, 2))
                t0a[:, g, 64:128, j, :] = np.transpose(
                    INf[(a + 2 * m + 1) % 64], (1, 0, 2))
        t0h = np.ascontiguousarray(
            np.transpose(INf[(a + 32) % 64], (1, 0, 2)))      # [NCH, 64, CH]
        # layer-1/2 tables (shared between the two layers)
        t1r = np.transpose(INf.reshape(NSL, GRP, NCH, CH), (2, 0, 1, 3))
        t1a = np.empty((NCH, NSL, 128, GRP, CH), BF16)
        t1a[:, :] = t1r[:, :, None, :, :]
        in_maps.append({"inp": INs, "tab0": t0a, "tab0h": t0h, "tab1": t1a,
                        **shared})
    return in_maps


def gather_out(results):
    return np.concatenate(
        [np.asarray(r["out"], np.float32).transpose(2, 0, 1).reshape(BL, 3 * O)
         for r in results], axis=0)


def kernel(input, W0, b0, W1, b1, W2, b2):
    if "nc" not in _cache:
        _cache["nc"] = _build_program()
    nc = _cache["nc"]
    in_maps = make_in_maps(input, W0, b0, W1, b1, W2, b2)
    res = run_bass_kernel_spmd(nc, in_maps, list(range(NCORES)))
    return gather_out(res.results)


# revision 20
# speedup vs baseline: 1.2133x; 1.2133x over previous
"""CIN (xDeepFM) 3-layer kernel for Trainium2, 8-core data parallel. v7.

Math (per layer l, with IN = input viewed [F=64, n] and X = previous
activation [H, n], n = (b, d) flattened):
    pre[o, n] = sum_{h, f} Wl[o, h, f] * X[h, n] * IN[f, n]
    Xnext = relu(pre + bl);  out_l[o, b] = sum_d Xnext[o, (b, d)]

v9 changes vs v7/v8:
  - bf16 d-sum accumulators: the 12 DVE tensor_reduce ops previously
    wrote f32, whose 4-byte output operand drops the DVE to 1x mode;
    bf16 output keeps them in 2x and off the critical path.  Converted
    to f32 once at the end (rel err 0.0021 -> 0.0036, vs 2e-2 gate).
    Interleaved same-session A/B at R=4097: 208.7us/iter vs v8's 252.6
    (-17%).

v7 changes vs v2 (190us):
  - Software-pipelined chunk order: L0(c+1) is emitted between L1(c)
    and L2(c).  In v2 the DVE idles at every layer boundary waiting for
    relu (last z8 -> 8 MMs -> ACT relu -> next layer's first z8); with
    the reorder, L0(c+1)'s independent z-builds (x0-based) fill the
    L1(c)->L2(c) bubble, and x1(c+1) is already computed when L2(c)
    ends, removing the L2(c)->L1(c+1) bubble entirely (~2 bubbles x
    ~2us x 4 chunks per pass).
  - Deeper table prefetch (tabbufs 13) and one more z buffer; the dead
    v2 square-path machinery (iden/insq/sel8/cw1/cw2) is removed to pay
    for the SBUF.
  - Layer 0 keeps the symmetric mod-64 diagonal tiling (K 4096 -> 2112,
    17 k-tiles of which one is K=64) with host-packed tables.
"""

import numpy as np
import ml_dtypes

import concourse.bass as bass
import concourse.bacc as bacc
import concourse.tile as tile
import concourse.mybir as mybir
from concourse.bass_utils import run_bass_kernel_spmd

BF16 = ml_dtypes.bfloat16

B, F, D = 512, 64, 32
NCORES = 8
BL = B // NCORES          # 64 batches per core
N = BL * D                # 2048 columns per core
CH = 512                  # chunk width (columns)
NCH = N // CH             # 4 chunks
O = 128                   # out channels per layer
GRP = 8                   # table rows per slot tile
NSL = F // GRP            # slots per layer 1/2 = 8
NT0 = 16                  # full layer-0 k-tiles (plus one K=64 tail)
bf16 = mybir.dt.bfloat16
f32 = mybir.dt.float32

_cache = {}


def _build_program(bench_repeat=None, zbufs=4, xcbufs=5, tabbufs=13):
    from contextlib import ExitStack, nullcontext

    nc = bacc.Bacc("TRN2")
    inp = nc.declare_dram_parameter("inp", [2 * F, N], bf16, isOutput=False)
    w0 = nc.declare_dram_parameter("w0", [128, NT0, 128], bf16, isOutput=False)
    w0h = nc.declare_dram_parameter("w0h", [64, 128], bf16, isOutput=False)
    w1 = nc.declare_dram_parameter("w1", [128, F, 128], bf16, isOutput=False)
    w2 = nc.declare_dram_parameter("w2", [128, F, 128], bf16, isOutput=False)
    b0 = nc.declare_dram_parameter("b0", [128, 1], f32, isOutput=False)
    b1 = nc.declare_dram_parameter("b1", [128, 1], f32, isOutput=False)
    b2 = nc.declare_dram_parameter("b2", [128, 1], f32, isOutput=False)
    # tables: layer-0 sym tiles (16 full in 2 GRP-slots + one 64-row tail),
    # layer-1/2 f-row slots (shared between the two layers)
    tab0 = nc.declare_dram_parameter("tab0", [NCH, 2, 128, GRP, CH], bf16,
                                     isOutput=False)
    tab0h = nc.declare_dram_parameter("tab0h", [NCH, 64, CH], bf16,
                                      isOutput=False)
    tab1 = nc.declare_dram_parameter("tab1", [NCH, NSL, 128, GRP, CH], bf16,
                                     isOutput=False)
    out = nc.declare_dram_parameter("out", [3, 128, BL], f32, isOutput=True)

    with tile.TileContext(nc) as tc, ExitStack() as ctx, \
            nc.allow_low_precision(reason="bf16 d-sum accumulators: 32-term "
                                   "sums of O(1) relu outputs, ~0.3% worst "
                                   "output rounding vs the 2e-2 gate"):
        wpool = ctx.enter_context(tc.tile_pool(name="w", bufs=1))
        xpool = ctx.enter_context(tc.tile_pool(name="x0", bufs=1))
        xc_pool = ctx.enter_context(tc.tile_pool(name="xc", bufs=xcbufs))
        tabs = ctx.enter_context(tc.tile_pool(name="tabs", bufs=tabbufs))
        zpool = ctx.enter_context(tc.tile_pool(name="z", bufs=zbufs))
        opool = ctx.enter_context(tc.tile_pool(name="oacc", bufs=1))
        pspool = ctx.enter_context(tc.tile_pool(name="ps", bufs=3, space="PSUM"))

        # resident weights / constants.  Load order matters for the
        # single-pass latency: x0 + layer-0 weights first (needed by the
        # first compute), the 4MB of w1/w2 after (first needed ~10us in).
        x0_t = xpool.tile([128, N], bf16)
        nc.sync.dma_start(x0_t[:], inp[:])
        w0_t = wpool.tile([128, NT0, 128], bf16)
        nc.sync.dma_start(w0_t[:], w0[:])
        w0h_t = wpool.tile([64, 128], bf16)
        nc.sync.dma_start(w0h_t[:], w0h[:])
        bias_ts = []
        for nm, bd in (("b0", b0), ("b1", b1), ("b2", b2)):
            bt = wpool.tile([128, 1], f32, name=nm)
            nc.sync.dma_start(bt[:], bd[:])
            bias_ts.append(bt)
        # issue via the ACT engine's DGE: separate DMA queue from the
        # (SP-issued) chunk tables, so table DMAs are not queued behind
        # 4MB of weights at single-pass start
        w1_t = wpool.tile([128, F, 128], bf16)
        nc.scalar.dma_start(w1_t[:], w1[:])
        w2_t = wpool.tile([128, F, 128], bf16)
        nc.scalar.dma_start(w2_t[:], w2[:])

        oacc = [opool.tile([128, BL], bf16, name=f"oacc{i}", tag=f"oacc{i}")
                for i in range(3)]
        oaccf = opool.tile([128, BL], f32, name="oaccf", tag="oaccf")

        def load_tables(c):
            t0 = []
            for g in range(2):
                s = tabs.tile([128, GRP, CH], bf16, tag="tab", name="s")
                nc.sync.dma_start(s[:], tab0[c, g])
                t0.append(s)
            t0h = tabs.tile([64, CH], bf16, tag="tabh", name="t0h")
            nc.sync.dma_start(t0h[:], tab0h[c])
            t1 = []
            for g in range(NSL):
                s = tabs.tile([128, GRP, CH], bf16, tag="tab", name="s")
                nc.sync.dma_start(s[:], tab1[c, g])
                t1.append(s)
            return t0, t0h, t1

        def emit_l0(c, tbl):
            t0, t0h, _ = tbl
            ns = c * CH
            bsl = c * (CH // D)
            ps0 = pspool.tile([128, CH], f32, tag="ps", name="ps0")
            for g in range(2):
                z8 = zpool.tile([128, GRP, CH], bf16, tag="z", name="z8")
                nc.vector.tensor_mul(
                    z8[:], x0_t[:, ns:ns + CH].unsqueeze(1)
                    .broadcast_to([128, GRP, CH]), t0[g][:])
                for j in range(GRP):
                    m = g * GRP + j
                    nc.tensor.matmul(ps0[:], w0_t[:, m, :], z8[:, j, :],
                                     start=(m == 0), stop=False)
            zh = zpool.tile([64, CH], bf16, tag="zh", name="zh")
            nc.vector.tensor_mul(zh[:], x0_t[0:64, ns:ns + CH], t0h[:])
            nc.tensor.matmul(ps0[:], w0h_t[:], zh[:], start=False, stop=True)

            x1c = xc_pool.tile([128, CH], bf16, tag="xc", name="x1c")
            nc.scalar.activation(x1c[:], ps0[:],
                                 mybir.ActivationFunctionType.Relu,
                                 bias=bias_ts[0], scale=1.0)
            nc.vector.tensor_reduce(
                oacc[0][:, bsl:bsl + CH // D],
                x1c.rearrange("p (g d) -> p g d", d=D),
                axis=mybir.AxisListType.X, op=mybir.AluOpType.add)
            return x1c

        def emit_layer(li, c, xin, tbl):
            t1 = tbl[2]
            bsl = c * (CH // D)
            w_t = w1_t if li == 1 else w2_t
            ps = pspool.tile([128, CH], f32, tag="ps", name="ps")
            for g in range(NSL):
                z8 = zpool.tile([128, GRP, CH], bf16, tag="z", name="z8")
                nc.vector.tensor_mul(
                    z8[:], xin[:].unsqueeze(1)
                    .broadcast_to([128, GRP, CH]), t1[g][:])
                for j in range(GRP):
                    f = g * GRP + j
                    nc.tensor.matmul(ps[:], w_t[:, f, :], z8[:, j, :],
                                     start=(f == 0), stop=(f == F - 1))
            xo = xc_pool.tile([128, CH], bf16, tag="xc", name="xo")
            nc.scalar.activation(xo[:], ps[:],
                                 mybir.ActivationFunctionType.Relu,
                                 bias=bias_ts[li], scale=1.0)
            nc.vector.tensor_reduce(
                oacc[li][:, bsl:bsl + CH // D],
                xo.rearrange("p (g d) -> p g d", d=D),
                axis=mybir.AxisListType.X, op=mybir.AluOpType.add)
            return xo

        loop_cm = tc.For_i(0, bench_repeat, 1) if bench_repeat else nullcontext()
        with loop_cm:
            tbl = load_tables(0)
            x1 = emit_l0(0, tbl)
            for c in range(NCH):
                tbl_next = load_tables(c + 1) if c + 1 < NCH else None
                x2 = emit_layer(1, c, x1, tbl)
                # L0(c+1) between L1(c) and L2(c): its x0-based z-builds
                # keep the DVE busy during L1(c)'s MM/relu tail, and
                # x1(c+1) is ready before L2(c) ends
                x1 = emit_l0(c + 1, tbl_next) if tbl_next else None
                emit_layer(2, c, x2, tbl)
                tbl = tbl_next

            for li in range(3):
                nc.scalar.activation(oaccf[:], oacc[li][:],
                                     mybir.ActivationFunctionType.Copy,
                                     bias=0.0, scale=1.0)
                nc.sync.dma_start(out[li], oaccf[:])

    nc.finalize()
    return nc


def _pack_weights(W0, b0, W1, b1, W2, b2):
    O_, F_ = 128, 64
    W0r = np.asarray(W0, np.float32).reshape(O_, F_, F_)   # [o, h, f]
    SW0 = W0r + W0r.transpose(0, 2, 1)

    # layer 0: tile m (0..15) packs groups t=2m (p<64) and t=2m+1 (p>=64);
    # tail tile = group t=32 at half weight. weight[p, m, o].
    a = np.arange(64)
    w0p = np.empty((128, NT0, O_), np.float32)
    for m in range(NT0):
        for half, t in ((0, 2 * m), (1, 2 * m + 1)):
            f = (a + t) % 64
            wv = SW0[:, a, f]                    # [o, 64]
            if t == 0:
                wv = wv / 2                      # diag counted twice in SW0
            w0p[half * 64:half * 64 + 64, m, :] = wv.T
    fh = (a + 32) % 64
    w0h = (SW0[:, a, fh] / 2).T                  # [64, o]

    def pack_l(W):
        Wr = np.asarray(W, np.float32).reshape(O_, 128, F_)   # [o, h, f]
        return np.ascontiguousarray(Wr.transpose(1, 2, 0)).astype(BF16)

    return {
        "w0": w0p.astype(BF16), "w0h": w0h.astype(BF16),
        "w1": pack_l(W1), "w2": pack_l(W2),
        "b0": np.asarray(b0, np.float32).reshape(128, 1),
        "b1": np.asarray(b1, np.float32).reshape(128, 1),
        "b2": np.asarray(b2, np.float32).reshape(128, 1),
    }


def make_in_maps(input, W0, b0, W1, b1, W2, b2):
    shared = _pack_weights(W0, b0, W1, b1, W2, b2)
    a = np.arange(64)
    in_maps = []
    inp_np = np.asarray(input)
    for core in range(NCORES):
        shard = inp_np[core * BL:(core + 1) * BL]          # [BL, F, D]
        IN = np.ascontiguousarray(
            shard.transpose(1, 0, 2).reshape(F, N)).astype(BF16)
        INs = np.ascontiguousarray(np.concatenate([IN, IN], axis=0))
        INf = IN.reshape(F, NCH, CH)
        # layer-0 sym tables: tab0[c, g, p, j, n] = IN[(p%64 + t)%64, ...],
        # t = 2*(8g+j) + p//64
        t0a = np.empty((NCH, 2, 128, GRP, CH), BF16)
        for g in range(2):
            for j in range(GRP):
                m = g * GRP + j
                t0a[:, g, 0:64, j, :] = np.transpose(
                    INf[(a + 2 * m) % 64], (1, 0, 2))
                t0a[:, g, 64:128, j, :] = np.transpose(
                    INf[(a + 2 * m + 1) % 64], (1, 0, 2))
        t0h = np.ascontiguousarray(
            np.transpose(INf[(a + 32) % 64], (1, 0, 2)))      # [NCH, 64, CH]
        # layer-1/2 tables (shared between the two layers)
        t1r = np.transpose(INf.reshape(NSL, GRP, NCH, CH), (2, 0, 1, 3))
        t1a = np.empty((NCH, NSL, 128, GRP, CH), BF16)
        t1a[:, :] = t1r[:, :, None, :, :]
        in_maps.append({"inp": INs, "tab0": t0a, "tab0h": t0h, "tab1": t1a,
                        **shared})
    return in_maps


def gather_out(results):
    return np.concatenate(
        [np.asarray(r["out"], np.float32).transpose(2, 0, 1).reshape(BL, 3 * O)
         for r in results], axis=0)


def kernel(input, W0, b0, W1, b1, W2, b2):
    if "nc" not in _cache:
        _cache["nc"] = _build_program()
    nc = _cache["nc"]
    in_maps = make_in_maps(input, W0, b0, W1, b1, W2, b2)
    res = run_bass_kernel_spmd(nc, in_maps, list(range(NCORES)))
    return gather_out(res.results)
